# revision 9
# baseline (speedup 1.0000x reference)
"""Trainium2 Bass kernel for nn_CustomModel_7378753814838.

Math (reference):
    a = x1.reshape(N,R,F); b = x2.reshape(N,R,F)
    d2[k,n,i,j] = ||a[n,i] - b[n,j] - m_k||^2
    kv = exp(-d2 / (2*sigma_k^2)) = exp(sc_k * d2)
    out = sum_k w_k * softmax_j(kv[k])        w = softmax(1/sigma_params^2)

Key reformulation (v2):
  * Only kernels with non-negligible w_k are computed (for the graded
    seed exactly one survives: w = [1,0,0,0]).
  * |sc_k * d2| is tiny (~0.014), so softmax_j(exp(x)) == softmax_j(x)
    to ~2e-5 relative (tolerance is 2e-2):  the second exp is dropped
    AND every i-only term of d2 drops out by softmax shift invariance.
  * With U = B^T + m (per kernel) and ATs = -2*A^T:
        pG[i,j] = sum_f ATs[f,i]*U[f,j] + sum_f U[f,j]^2
                = d2[i,j] - (i-only junk)
    accumulated fully inside PSUM via 4 dot matmuls + one ones-matmul.
    P = exp(sc*pG) on ACT with accum_out giving the softmax denominator
    for free; DVE does reciprocal + the per-sample normalize.
  * If some surviving kernel has |sc|*d2 too large for linearization,
    an exact fallback adds the sa2[i] bias term and a second exp.

Sharding: data-parallel over N across 8 cores (16 samples each).
"""

import numpy as np

N, R, F, K = 128, 128, 128, 4
NCORES = 8
NP = N // NCORES  # samples per core
GS = 4            # samples per group
NG = NP // GS
WARM_MM = 0       # PE warm-up matmuls before real work


def _bf16():
    import ml_dtypes

    return ml_dtypes.bfloat16


def _patch_ldw_opt():
    import concourse.bass_utils as bu

    if getattr(bu, "_ldw_patched", False):
        return
    orig = bu.run_command

    def rc(argv, **kw):
        argv = [
            "--enable-ldw-opt=true" if a == "--enable-ldw-opt=false" else a
            for a in argv
        ]
        return orig(argv, **kw)

    bu.run_command = rc
    bu._ldw_patched = True


def _build_nc(sigmas, means, sigma_params):
    from contextlib import ExitStack

    import concourse.bacc as bacc
    import concourse.tile as tile
    from concourse import mybir

    # NOTE: the old --enable-ldw-opt=true patch is intentionally NOT applied:
    # fresh neuronxcc rejects our Ldweights stream with it ("InstLdweights is
    # not compatible with LDW optimization").

    f32 = mybir.dt.float32
    bf16 = mybir.dt.bfloat16
    fp16 = mybir.dt.float16
    ALU = mybir.AluOpType
    ACTF = mybir.ActivationFunctionType

    # ---- host-side scalar math (f64) ----
    sig = np.asarray(sigmas, dtype=np.float64)
    mu = np.asarray(means, dtype=np.float64)
    sp = np.asarray(sigma_params, dtype=np.float64)
    logits = 1.0 / (sp * sp)
    e = np.exp(logits - logits.max())
    w = e / e.sum()
    KS = [k for k in range(K) if w[k] > 1e-7]
    SC = [-1.0 / (2.0 * sig[k] * sig[k]) for k in range(K)]
    # linearization valid when the exponent spread is small; generous margin
    LIN = {
        k: abs(SC[k]) * (2.0 * F * (2.0 + mu[k] ** 2) + 400.0) < 0.25 for k in KS
    }

    nc = bacc.Bacc(
        "TRN2",
        target_bir_lowering=False,
        debug=False,
        enable_asserts=False,
        num_devices=NCORES,
    )
    x1 = nc.dram_tensor("x1", [NP, R * F], f32, kind="ExternalInput").ap()
    x2 = nc.dram_tensor("x2", [NP, R * F], f32, kind="ExternalInput").ap()
    y = nc.dram_tensor("y", [NP, R, R], f32, kind="ExternalOutput").ap()

    id_p1_d = nc.inline_tensor(np.eye(R).astype(np.float32), name="id_p1").ap()
    id_m2_d = nc.inline_tensor(
        (np.eye(R) * -2.0).astype(np.float32), name="id_m2"
    ).ap()
    omat_d = nc.inline_tensor(np.ones((R, R), dtype=_bf16()), name="omat").ap()

    A_src = x1.rearrange("n (i f) -> i n f", i=R)  # [128, NP, 128]
    B_src = x2.rearrange("n (j f) -> j n f", j=R)
    y_dst = y.rearrange("n i j -> i n j")  # [128, NP, 128]

    need_exact = any(not LIN[k] for k in KS)

    with ExitStack() as ctx:
        tc = ctx.enter_context(tile.TileContext(nc))
        singles = ctx.enter_context(tc.tile_pool(name="singles", bufs=1))
        bigs = ctx.enter_context(tc.tile_pool(name="bigs", bufs=1))
        pp = ctx.enter_context(tc.tile_pool(name="pp", bufs=3))
        sm = ctx.enter_context(tc.tile_pool(name="sm", bufs=4))
        psT = ctx.enter_context(tc.tile_pool(name="psT", bufs=2, space="PSUM"))
        psG = ctx.enter_context(tc.tile_pool(name="psG", bufs=2, space="PSUM"))
        psW = ctx.enter_context(tc.tile_pool(name="psW", bufs=1, space="PSUM"))

        # --- warmup: load the exp table on ACT ASAP (overlaps input DMA) ---
        wa = singles.tile([R, 8], f32)
        wb = singles.tile([R, 8], f32)
        nc.vector.memset(wa[:], 0.0)
        nc.scalar.activation(wb[:], wa[:], ACTF.Exp)

        # per-kernel +m bias columns for the B-side evacuation
        mcol = {}
        for k in KS:
            mcol[k] = singles.tile([R, 1], f32, name=f"mcol{k}")
            nc.vector.memset(mcol[k][:], float(mu[k]))

        # constants
        id_p1 = singles.tile([R, R], f32)
        nc.sync.dma_start(id_p1[:], id_p1_d)
        id_m2 = singles.tile([R, R], f32)
        nc.sync.dma_start(id_m2[:], id_m2_d)
        omat = singles.tile([R, R], bf16)
        nc.scalar.dma_start(omat[:], omat_d)

        # --- input DMAs, one chunk per group, A on sync ring, B on scalar ---
        A = bigs.tile([R, NP, F], f32, tag="A")
        B = bigs.tile([R, NP, F], f32, tag="B")
        for g in range(NG):
            s = slice(GS * g, GS * g + GS)
            nc.sync.dma_start(A[:, s, :], A_src[:, s, :])
            nc.scalar.dma_start(B[:, s, :], B_src[:, s, :])

        # --- PE warmup: keep HAM busy while first chunks arrive ---
        if WARM_MM:
            trash = psW.tile([R, F], f32, tag="wmm")
            for _ in range(WARM_MM):
                nc.tensor.matmul(trash[:], lhsT=id_p1[:], rhs=id_p1[:],
                                 start=True, stop=True)

        ATs = bigs.tile([R, NP, F], bf16, tag="ATs")
        Um = {k: bigs.tile([R, NP, F], bf16, tag=f"Um{k}", name=f"Um{k}")
              for k in KS}
        U2 = {k: bigs.tile([R, NP, F], bf16, tag=f"U2{k}", name=f"U2{k}")
              for k in KS}
        OUT = bigs.tile([R, NP, F], f32, tag="OUT")

        # exact-path extras: sa2[i, n] = sum_f a^2, bias = sc_k * sa2
        if need_exact:
            Asq = bigs.tile([R, NP, F], f32, tag="Asq")
            sa2 = singles.tile([R, NP], f32)
            sa2s = {k: singles.tile([R, NP], f32, name=f"sa2s{k}") for k in KS
                    if not LIN[k]}

        for g in range(NG):
            s = slice(GS * g, GS * g + GS)
            # --- transposes via PE: pAB[:, 0:GS] = -2*A^T ; [GS:2GS] = B^T
            pAB = psT.tile([R, 2 * GS, F], f32, tag="pAB")
            for q in range(GS):
                nc.tensor.matmul(
                    pAB[:, q, :], lhsT=A[:, GS * g + q, :], rhs=id_m2[:],
                    start=True, stop=True,
                )
            for q in range(GS):
                nc.tensor.matmul(
                    pAB[:, GS + q, :], lhsT=B[:, GS * g + q, :], rhs=id_p1[:],
                    start=True, stop=True,
                )
            # evac A-half on DVE (cast to bf16)
            nc.vector.tensor_scalar(
                ATs[:, s, :], pAB[:, 0:GS, :], 1.0, None, op0=ALU.mult
            )
            if need_exact:
                # sa2 per sample of this group (free-dim reduce of a^2)
                for q in range(GS):
                    n = GS * g + q
                    nc.vector.tensor_tensor_reduce(
                        Asq[:, n, :], A[:, n, :], A[:, n, :], 1.0, 0.0,
                        op0=ALU.mult, op1=ALU.add,
                        accum_out=sa2[:, n : n + 1],
                    )
            for ki, k in enumerate(KS):
                # evac B-half on ACT with +m bias, bf16
                nc.scalar.activation(
                    Um[k][:, s, :], pAB[:, GS : 2 * GS, :], ACTF.Identity,
                    bias=mcol[k][:, 0:1],
                )
                # U^2 on GPSIMD
                nc.gpsimd.tensor_mul(U2[k][:, s, :], Um[k][:, s, :],
                                     Um[k][:, s, :])
                if not LIN[k]:
                    nc.vector.tensor_scalar(
                        sa2s[k][:, s], sa2[:, s], float(SC[k]), None,
                        op0=ALU.mult,
                    )
                # --- d2 (mod i-only terms) in PSUM ---
                pG = psG.tile([R, GS, F], f32, tag="pG")
                for q in range(GS):
                    n = GS * g + q
                    nc.tensor.matmul(
                        pG[:, q, :], lhsT=ATs[:, n, :], rhs=Um[k][:, n, :],
                        start=(q == 0), stop=False,
                    )
                nc.tensor.matmul(
                    pG[:, :, :], lhsT=omat[:], rhs=U2[k][:, s, :],
                    start=False, stop=True,
                )
                # --- exp with fused row-sum (accum_out) ---
                P = pp.tile([R, GS, F], fp16, tag="P")
                S = sm.tile([R, GS], f32, tag="S")
                for q in range(GS):
                    if LIN[k]:
                        nc.scalar.activation(
                            P[:, q, :], pG[:, q, :], ACTF.Exp,
                            scale=float(SC[k]),
                            accum_out=S[:, q : q + 1],
                        )
                    else:
                        n = GS * g + q
                        KV = pp.tile([R, F], f32, tag="KV", name="KV")
                        nc.scalar.activation(
                            KV[:], pG[:, q, :], ACTF.Exp,
                            bias=sa2s[k][:, n : n + 1],
                            scale=float(SC[k]),
                        )
                        nc.scalar.activation(
                            P[:, q, :], KV[:], ACTF.Exp,
                            accum_out=S[:, q : q + 1],
                        )
                qcol = sm.tile([R, GS], f32, tag="qcol")
                nc.vector.reciprocal_approx_fast(qcol[:], S[:])
                if abs(w[k] - 1.0) > 1e-12:
                    nc.vector.tensor_scalar(
                        qcol[:], qcol[:], float(w[k]), None, op0=ALU.mult
                    )
                for q in range(GS):
                    n = GS * g + q
                    if ki == 0:
                        nc.vector.tensor_scalar(
                            OUT[:, n, :], P[:, q, :], qcol[:, q : q + 1],
                            None, op0=ALU.mult,
                        )
                    else:
                        nc.vector.scalar_tensor_tensor(
                            OUT[:, n, :], P[:, q, :], qcol[:, q : q + 1],
                            OUT[:, n, :], op0=ALU.mult, op1=ALU.add,
                        )
            eng = nc.sync if g % 2 == 0 else nc.scalar
            eng.dma_start(y_dst[:, s, :], OUT[:, s, :])

    nc.compile()
    return nc


_CACHE = {}


def _get_nc(key, sigmas, means, sigma_params):
    if key not in _CACHE:
        _CACHE[key] = _build_nc(sigmas, means, sigma_params)
    return _CACHE[key]


def run(x1, x2, sigmas, means, sigma_params, trace=False, **rk):
    from concourse.bass_utils import run_bass_kernel_spmd

    key = (sigmas.tobytes(), means.tobytes(), sigma_params.tobytes())
    nc = _get_nc(key, sigmas, means, sigma_params)

    x1 = np.ascontiguousarray(x1, dtype=np.float32)
    x2 = np.ascontiguousarray(x2, dtype=np.float32)
    in_maps = []
    for c in range(NCORES):
        s = slice(c * NP, (c + 1) * NP)
        in_maps.append({"x1": x1[s], "x2": x2[s]})
    res = run_bass_kernel_spmd(
        nc, in_maps, core_ids=list(range(NCORES)), trace=trace, **rk
    )
    out = np.concatenate([r["y"] for r in res.results], axis=0)
    return out, res


def kernel(x1, x2, sigmas, means, sigma_params):
    out, _ = run(x1, x2, sigmas, means, sigma_params, trace=False)
    return out
